# revision 22
# baseline (speedup 1.0000x reference)
"""Transformer-XL CompressiveLayer on 8 TRN2 NeuronCores.

Sharding: core c = (batch b = c//2) x (head-half hh = c%2).
Each core handles one batch's full 896 tokens with 8 of 16 heads and
2048 of 4096 FFN inner channels.

vs the earlier AllReduce design: the cores in a pair exchange the much
smaller attn_vec halves (bf16, 0.92MB) via four per-feature-chunk
AllGathers that overlap the per-head attention loop; each core then
computes the full o_proj + LN2 locally.  LN1 and the (batch-invariant)
r-projection are computed on the host, which removes the serial LN1 /
transpose prologue from the device entirely.  The BD+AC add runs on the
PE via an identity-matmul accumulate instead of a DVE tensor_add, and
the rel-shift pad write carries its zero column inside the tile (one
contiguous DMA, no strided zero fill).  FFN partial outputs are summed
on the host (plus ff2 bias).

All matmuls run in bf16 with fp32 PSUM accumulation; softmax and LN2
run in fp32.  rel_shift is realized exactly via a padded DRAM buffer
per head: BD tiles (with zero col 0) are written at flat offset
897*(128*it) and the shifted matrix is read back at flat offset
896 + 896*(128*it).
"""

import numpy as np
import ml_dtypes
from contextlib import ExitStack

import concourse.bass as bass
import concourse.tile as tile
from concourse import mybir, bacc
from concourse.masks import make_identity

F32 = mybir.dt.float32
BF16 = mybir.dt.bfloat16
BF = ml_dtypes.bfloat16

D, H, DH, FF = 1024, 16, 64, 4096
S, B, M, CM = 512, 4, 256, 128
K = S + M + CM          # 896 tokens
NT = K // 128           # 7 token tiles
ND = D // 128           # 8 D chunks
HC = 8                  # heads per core
FC = HC * DH            # 512 features per core
NFC = FC // 128         # 4 feature chunks per core
NFG = 2 * NFC           # 8 global feature chunks
FFC = FF // 2           # 2048 ffn channels per core
NFF = FFC // 128        # 16 ffn chunks per core
SCALE = 1.0 / np.sqrt(DH)
EPS = 1e-5

_CACHED = {}


def _ln_tile(nc, x_t, eps_sb, stat_pool):
    """In: x_t [128,1024] f32. Returns (mean, rstd) [128,1] tiles."""
    stats = stat_pool.tile([128, 2, 6], F32, tag="stats")
    for g in range(2):
        nc.vector.bn_stats(out=stats[:, g, :], in_=x_t[:, g * 512:(g + 1) * 512])
    mv = stat_pool.tile([128, 2], F32, tag="mv")
    nc.vector.bn_aggr(out=mv, in_=stats)
    rstd = stat_pool.tile([128, 1], F32, tag="rstd")
    nc.scalar.activation(out=rstd, in_=mv[:, 1:2],
                         func=mybir.ActivationFunctionType.Sqrt,
                         bias=eps_sb, scale=1.0)
    nc.vector.reciprocal(out=rstd, in_=rstd)
    return mv[:, 0:1], rstd


def build(ln2_trivial=True, debug=False, sim_mode=False):
    nc = bacc.Bacc(None)

    wT_p = nc.declare_dram_parameter("wT", [128, NT, ND, 128], BF16,
                                     isOutput=False)
    wres_p = nc.declare_dram_parameter("wres", [K, D], BF16, isOutput=False)
    rT_p = nc.declare_dram_parameter("rT", [128, NFC, K], BF16, isOutput=False)
    qw = nc.declare_dram_parameter("qw", [128, ND, FC], BF16, isOutput=False)
    kw = nc.declare_dram_parameter("kw", [128, ND, FC], BF16, isOutput=False)
    vw = nc.declare_dram_parameter("vw", [128, ND, FC], BF16, isOutput=False)
    ow = nc.declare_dram_parameter("ow", [128, NFG, D], BF16, isOutput=False)
    ff1w = nc.declare_dram_parameter("ff1w", [128, ND, FFC], BF16,
                                     isOutput=False)
    ff2w = nc.declare_dram_parameter("ff2w", [128, NFF, D], BF16,
                                     isOutput=False)
    rwb = nc.declare_dram_parameter("rwb", [128, NFC], F32, isOutput=False)
    rrb = nc.declare_dram_parameter("rrb", [128, NFC], F32, isOutput=False)
    ff1b = nc.declare_dram_parameter("ff1b", [128, NFF], F32, isOutput=False)
    if not ln2_trivial:
        ln2w = nc.declare_dram_parameter("ln2w", [D], F32, isOutput=False)
        ln2b = nc.declare_dram_parameter("ln2b", [D], F32, isOutput=False)

    out = nc.declare_dram_parameter("out", [K, D], F32, isOutput=True)

    pads = [nc.dram_tensor(f"pad{i}", [K * (K + 1)], BF16) for i in range(HC)]
    ag_in = nc.dram_tensor("ag_in", [NFC * 128 * K], BF16)
    ag_out = nc.dram_tensor("ag_out", [2 * NFC * 128 * K], BF16)

    with tile.TileContext(nc) as tc, ExitStack() as ctx:
        consts = ctx.enter_context(tc.tile_pool(name="consts", bufs=1))
        psS = ctx.enter_context(tc.tile_pool(name="psS", bufs=2, space="PSUM"))
        stat_pool = ctx.enter_context(tc.tile_pool(name="stats", bufs=8))

        eps_sb = consts.tile([128, 1], F32)
        nc.vector.memset(eps_sb, EPS)
        ident = consts.tile([128, 128], BF16, tag="ident")
        make_identity(nc, ident)
        rwb_sb = consts.tile([128, NFC], F32, tag="rwb")
        nc.sync.dma_start(out=rwb_sb, in_=rwb[:])
        rrb_sb = consts.tile([128, NFC], F32, tag="rrb")
        nc.sync.dma_start(out=rrb_sb, in_=rrb[:])
        ff1b_sb = consts.tile([128, NFF], F32, tag="ff1b")
        nc.sync.dma_start(out=ff1b_sb, in_=ff1b[:])
        if not ln2_trivial:
            def bcast(name, param):
                t = consts.tile([128, D], F32, tag=name)
                nc.sync.dma_start(out=t, in_=bass.AP(tensor=param, offset=0,
                                                     ap=[[0, 128], [1, D]]))
                return t
            ln2w_bc = bcast("ln2w_bc", ln2w)
            ln2b_bc = bcast("ln2b_bc", ln2b)

        # late weights: pW1 lives from the start; pW2 is created only after
        # the attention pools free their SBUF
        pW1 = ctx.enter_context(tc.tile_pool(name="pW1", bufs=1))
        ow_sb = pW1.tile([128, NFG, D], BF16, tag="ow")
        wres_sb = pW1.tile([128, NT, D], BF16, tag="wres")
        ff1w_sb = pW1.tile([128, ND, FFC], BF16, tag="ff1w")

        avp = ctx.enter_context(tc.tile_pool(name="avp", bufs=1))
        avA = [avp.tile([128, K], BF16, tag=f"av{f}", name=f"av{f}")
               for f in range(NFG)]

        # persistent activations for attention (closed before pW2 is created)
        atn = ExitStack()
        atnp = atn.enter_context(tc.tile_pool(name="atnp", bufs=1))
        q1T = atnp.tile([128, NFC, K], BF16, tag="q1T")
        q2T = atnp.tile([128, NFC, K], BF16, tag="q2T")
        kT = atnp.tile([128, NFC, K], BF16, tag="kT")
        rT_sb = atnp.tile([128, NFC, K], BF16, tag="rT")
        v1 = atnp.tile([128, NT, HC, DH], BF16, tag="v1")

        # ---------------- P0: load weights, projections ----------------
        with tc.tile_pool(name="p0w", bufs=1) as p0w:
            wT = p0w.tile([128, NT, ND, 128], BF16, tag="wT")
            nc.sync.dma_start(out=wT, in_=wT_p[:])
            qw_sb = p0w.tile([128, ND, FC], BF16, tag="qw")
            nc.scalar.dma_start(out=qw_sb, in_=qw[:])
            kw_sb = p0w.tile([128, ND, FC], BF16, tag="kw")
            nc.scalar.dma_start(out=kw_sb, in_=kw[:])
            vw_sb = p0w.tile([128, ND, FC], BF16, tag="vw")
            nc.scalar.dma_start(out=vw_sb, in_=vw[:])
            nc.scalar.dma_start(out=rT_sb, in_=rT_p[:])
            # o_w / residual loads, split to avoid long DMA-engine holds
            for fcg in range(NFG):
                nc.scalar.dma_start(out=ow_sb[:, fcg, :], in_=ow[:, fcg, :])
            for t in range(NT):
                nc.scalar.dma_start(
                    out=wres_sb[:, t, :],
                    in_=wres_p[t * 128:(t + 1) * 128, :])

            def projT(w_sb, writeback):
                for fc in range(NFC):
                    ps = psS.tile([128, 1024], F32, tag="s", name=f"psp{fc}")
                    for dc in range(ND):
                        st = dict(start=(dc == 0), stop=(dc == ND - 1))
                        lh = w_sb[:, dc, fc * 128:(fc + 1) * 128]
                        nc.tensor.matmul(ps[:, 0:512], lh, wT[:, 0:4, dc, :],
                                         **st)
                        nc.tensor.matmul(ps[:, 512:896], lh, wT[:, 4:7, dc, :],
                                         **st)
                    writeback(fc, ps)

            def q_wb(fc, ps):
                nc.vector.tensor_scalar_add(
                    out=q1T[:, fc, :], in0=ps[:, 0:K],
                    scalar1=rwb_sb[:, fc:fc + 1])
                nc.vector.tensor_scalar_add(
                    out=q2T[:, fc, :], in0=ps[:, 0:K],
                    scalar1=rrb_sb[:, fc:fc + 1])

            def k_wb(fc, ps):
                eng = nc.scalar.copy if fc % 2 else nc.vector.tensor_copy
                eng(out=kT[:, fc, :], in_=ps[:, 0:K])

            projT(qw_sb, q_wb)
            projT(kw_sb, k_wb)

            for t in range(NT):
                pv = psS.tile([128, 1024], F32, tag="s", name="psv")
                for dc in range(ND):
                    nc.tensor.matmul(pv[:, 0:512], wT[:, t, dc, :],
                                     vw_sb[:, dc, :],
                                     start=(dc == 0), stop=(dc == ND - 1))
                eng = nc.scalar.copy if t % 2 else nc.vector.tensor_copy
                eng(out=v1[:, t, :, :],
                    in_=pv[:, 0:512].rearrange("p (h d) -> p h d", h=HC))

        # ---------------- P1: per-head attention + avT exchange ----------
        # Software pipeline: PV of head h-1 is emitted between the BD and
        # AC/softmax phases of head h so its matmuls fill the pad-DMA and
        # softmax latency; the pair AllGather and its imports ride the
        # otherwise idle gpsimd queue so no compute queue waits on the
        # collective.
        with tc.tile_pool(name="pBD", bufs=1) as pBD, \
             tc.tile_pool(name="pSBD", bufs=2) as pSBD, \
             tc.tile_pool(name="pPX", bufs=2) as pPX, \
             tc.tile_pool(name="ptwp", bufs=2) as ptwp, \
             tc.tile_pool(name="psPV", bufs=1, space="PSUM") as psPV:
            ptws = {}
            bdts = {}
            sbds = {}
            pexps = {}

            def bd_phase_start(h):
                bdts[h] = pBD.tile([128, NT, K + 1], BF16, tag="bd",
                                   name="bd")
                nc.vector.memset(bdts[h][:, :, 0:1], 0.0)

            def bd_step(h, it):
                hp, fc = (h % 2) * DH, h // 2
                ps = psS.tile([128, 1024], F32, tag="s", name="psbd")
                nc.tensor.matmul(
                    ps[:, 0:512],
                    q2T[hp:hp + DH, fc, it * 128:(it + 1) * 128],
                    rT_sb[hp:hp + DH, fc, 0:512], start=True, stop=True)
                nc.tensor.matmul(
                    ps[:, 512:896],
                    q2T[hp:hp + DH, fc, it * 128:(it + 1) * 128],
                    rT_sb[hp:hp + DH, fc, 512:896], start=True, stop=True)
                eng = nc.scalar.copy if it % 2 else nc.vector.tensor_copy
                eng(out=bdts[h][:, it, 1:K + 1], in_=ps[:, 0:K])

            def bd_phase_finish(h):
                # one contiguous pad write for all 7 row tiles (zero col
                # included in the tile), then one strided read of the
                # shifted matrix
                nc.sync.dma_start(
                    out=bass.AP(tensor=pads[h], offset=0,
                                ap=[[(K + 1) * 128, NT], [K + 1, 128],
                                    [1, K + 1]]),
                    in_=bdts.pop(h))
                sbds[h] = pSBD.tile([128, NT, K], BF16, tag="sbd",
                                    name="sbd")
                nc.sync.dma_start(
                    out=sbds[h],
                    in_=bass.AP(tensor=pads[h], offset=K,
                                ap=[[K * 128, NT], [K, 128], [1, K]]))

            def s_step(h, it, sbd):
                hp, fc = (h % 2) * DH, h // 2
                ps = psS.tile([128, 1024], F32, tag="s", name="psac")
                nc.tensor.matmul(
                    ps[:, 0:512],
                    q1T[hp:hp + DH, fc, it * 128:(it + 1) * 128],
                    kT[hp:hp + DH, fc, 0:512], start=True, stop=False)
                nc.tensor.matmul(
                    ps[:, 0:512], ident, sbd[:, 0:512],
                    start=False, stop=True)
                nc.tensor.matmul(
                    ps[:, 512:896],
                    q1T[hp:hp + DH, fc, it * 128:(it + 1) * 128],
                    kT[hp:hp + DH, fc, 512:896], start=True, stop=False)
                nc.tensor.matmul(
                    ps[:, 512:896], ident, sbd[:, 512:896],
                    start=False, stop=True)
                pexp = pPX.tile([128, K], BF16, tag="px", name="px")
                acc = stat_pool.tile([128, 1], F32, tag="acc", name="acc")
                nc.scalar.activation(
                    out=pexp, in_=ps[:, 0:K],
                    func=mybir.ActivationFunctionType.Exp,
                    scale=float(SCALE), accum_out=acc)
                rcp = stat_pool.tile([128, 1], F32, tag="rcp", name="rcp")
                nc.vector.reciprocal(out=rcp, in_=acc)
                nc.vector.tensor_scalar_mul(out=pexp, in0=pexp,
                                            scalar1=rcp)
                nc.sync.dma_start_transpose(ptws[h][:, it, :, :], pexp)

            pv_state = {}

            def pv_start(h):
                pv_state[h] = (ptws.pop(h),
                               psPV.tile([64, 1024], F32, tag="pv",
                                         name="pvx"))

            def pv_step(h, jt):
                ptw, pvx = pv_state[h]
                st = dict(start=(jt == 0), stop=(jt == NT - 1))
                nc.tensor.matmul(pvx[:, 0:512], v1[:, jt, h, :],
                                 ptw[:, 0:4, jt, :], **st)
                nc.tensor.matmul(pvx[:, 512:896], v1[:, jt, h, :],
                                 ptw[:, 4:7, jt, :], **st)

            def pv_finish(h):
                hp, fc = (h % 2) * DH, h // 2
                _, pvx = pv_state.pop(h)
                eng = nc.scalar.copy if h % 2 else nc.vector.tensor_copy
                eng(out=avA[fc][hp:hp + DH, 0:512], in_=pvx[:, 0:512])
                eng(out=avA[fc][hp:hp + DH, 512:K], in_=pvx[:, 512:896])

            def pv(h):
                pv_start(h)
                for jt in range(NT):
                    pv_step(h, jt)
                pv_finish(h)

            def store_chunk(f):
                # export own chunk f to the AllGather staging buffer
                nc.sync.dma_start(
                    out=bass.AP(tensor=ag_in, offset=f * 128 * K,
                                ap=[[K, 128], [1, K]]),
                    in_=avA[f])

            def exchange_all():
                # one AllGather of all four chunks with the pair partner,
                # then import rank0's block into global slots 0-3 and
                # rank1's into 4-7 (symmetric on both cores)
                if sim_mode:
                    for r in range(2):
                        nc.gpsimd.dma_start(
                            out=bass.AP(tensor=ag_out,
                                        offset=r * NFC * 128 * K,
                                        ap=[[1, NFC * 128 * K]]),
                            in_=bass.AP(tensor=ag_in, offset=0,
                                        ap=[[1, NFC * 128 * K]]))
                else:
                    nc.gpsimd.collective_compute(
                        "AllGather", mybir.AluOpType.bypass,
                        replica_groups=[[0, 1], [2, 3], [4, 5], [6, 7]],
                        ins=[bass.AP(tensor=ag_in, offset=0,
                                     ap=[[1, NFC * 128 * K]])],
                        outs=[bass.AP(tensor=ag_out, offset=0,
                                      ap=[[1, 2 * NFC * 128 * K]])])
                for g in range(NFG):
                    eng = nc.sync if g % 2 else nc.scalar
                    eng.dma_start(
                        out=avA[g],
                        in_=bass.AP(tensor=ag_out, offset=g * 128 * K,
                                    ap=[[K, 128], [1, K]]))

            for h in range(HC):
                ptws[h] = ptwp.tile([128, NT, NT, 128], BF16, tag="ptw",
                                    name="ptw")
                bd_phase_start(h)
                for it in range(NT):
                    bd_step(h, it)
                bd_phase_finish(h)
                if h >= 1:
                    pv(h - 1)
                sbd_all = sbds.pop(h)
                for it in range(NT):
                    s_step(h, it, sbd_all[:, it, :])
                if h >= 2 and h % 2 == 0:
                    store_chunk(h // 2 - 1)
                if h == 6:
                    for dc in range(ND):
                        nc.scalar.dma_start(out=ff1w_sb[:, dc, :],
                                            in_=ff1w[:, dc, :])
            pv(HC - 1)
            store_chunk(NFC - 1)
            exchange_all()

        atn.close()
        # ---------------- P2: o_proj + LN2 ----------------
        pW2 = ctx.enter_context(tc.tile_pool(name="pW2", bufs=1))
        ff2w_sb = pW2.tile([128, NFF, D], BF16, tag="ff2w")
        for ffc in range(NFF):
            nc.scalar.dma_start(out=ff2w_sb[:, ffc, :], in_=ff2w[:, ffc, :])
        hT = pW2.tile([128, NFF, K], BF16, tag="hT")
        arT = pW2.tile([128, NT, ND, 128], BF16, tag="arT")

        with tc.tile_pool(name="p2", bufs=2) as p2:
            for it in range(NT):
                po = psS.tile([128, 1024], F32, tag="s", name="po")
                for fcg in range(NFG):
                    st = dict(start=(fcg == 0), stop=(fcg == NFG - 1))
                    l = avA[fcg][:, it * 128:(it + 1) * 128]
                    nc.tensor.matmul(po[:, 0:512], l, ow_sb[:, fcg, 0:512],
                                     **st)
                    nc.tensor.matmul(po[:, 512:1024], l,
                                     ow_sb[:, fcg, 512:1024], **st)
                x2 = p2.tile([128, D], F32, tag="x2")
                nc.vector.tensor_add(out=x2, in0=po, in1=wres_sb[:, it, :])
                mean, rstd = _ln_tile(nc, x2, eps_sb, stat_pool)
                ars = p2.tile([128, D], BF16, tag="ars")
                if ln2_trivial:
                    nc.vector.tensor_scalar(out=ars, in0=x2, scalar1=mean,
                                            scalar2=rstd,
                                            op0=mybir.AluOpType.subtract,
                                            op1=mybir.AluOpType.mult)
                else:
                    nc.vector.tensor_scalar(out=x2, in0=x2, scalar1=mean,
                                            scalar2=rstd,
                                            op0=mybir.AluOpType.subtract,
                                            op1=mybir.AluOpType.mult)
                    nc.gpsimd.tensor_mul(out=x2, in0=x2, in1=ln2w_bc)
                    nc.vector.tensor_add(out=ars, in0=x2, in1=ln2b_bc)
                nc.sync.dma_start_transpose(arT[:, it, :, :], ars)


        # ---------------- P3: FFN ----------------
        with tc.tile_pool(name="psB", bufs=2, space="PSUM") as psB:
            for half, (i0, n, sl) in enumerate(
                    [(0, 512, (0, 4)), (512, 384, (4, 7))]):
                for ffc in range(NFF):
                    ph = psB.tile([128, 512], F32, tag="h", name="ph")
                    for dc in range(ND):
                        st = dict(start=(dc == 0), stop=(dc == ND - 1))
                        lh = ff1w_sb[:, dc, ffc * 128:(ffc + 1) * 128]
                        nc.tensor.matmul(ph[:, 0:n], lh,
                                         arT[:, sl[0]:sl[1], dc, :], **st)
                    nc.scalar.activation(
                        out=hT[:, ffc, i0:i0 + n], in_=ph[:, 0:n],
                        func=mybir.ActivationFunctionType.Relu,
                        bias=ff1b_sb[:, ffc:ffc + 1], scale=1.0)
            with tc.tile_pool(name="p3", bufs=3) as p3:
                for it in range(NT):
                    po = psS.tile([128, 1024], F32, tag="s", name="pf")
                    for ffc in range(NFF):
                        st = dict(start=(ffc == 0), stop=(ffc == NFF - 1))
                        l = hT[:, ffc, it * 128:(it + 1) * 128]
                        nc.tensor.matmul(po[:, 0:512], l,
                                         ff2w_sb[:, ffc, 0:512], **st)
                        nc.tensor.matmul(po[:, 512:1024], l,
                                         ff2w_sb[:, ffc, 512:1024], **st)
                    ot = p3.tile([128, D], F32, tag="ot")
                    eng = nc.scalar.copy if it % 2 else nc.vector.tensor_copy
                    eng(out=ot, in_=po)
                    nc.sync.dma_start(out=out[it * 128:(it + 1) * 128, :],
                                      in_=ot)

    nc.finalize()
    return nc


def prep_inputs(inputs):
    """Full inputs -> list of 8 per-core input maps (host does LN1 + r-proj
    + all layout transposes)."""
    x_nat = np.concatenate([inputs["input_ids"], inputs["mem"],
                            inputs["c_mem"]], axis=0)  # [K,B,D] f32
    x_nat = np.asarray(x_nat, np.float32)
    mu = x_nat.mean(-1, keepdims=True)
    var = x_nat.var(-1, keepdims=True)
    w_all = ((x_nat - mu) / np.sqrt(var + EPS)
             * np.asarray(inputs["ln1_w"], np.float32)
             + np.asarray(inputs["ln1_b"], np.float32))  # [K,B,D] f32

    r_full = (np.asarray(inputs["positional_embedding"], np.float32)
              @ np.asarray(inputs["r_w"], np.float32))  # [K, H*DH] f32

    qkv = inputs["qkv_w"]
    ow_full = np.ascontiguousarray(
        np.asarray(inputs["o_w"], np.float32).astype(BF)
        .reshape(NFG, 128, D).transpose(1, 0, 2))

    maps = []
    for c in range(8):
        b, hh = c // 2, c % 2
        Fs = slice(hh * FC, (hh + 1) * FC)
        FFs = slice(hh * FFC, (hh + 1) * FFC)

        def wchunk(wmat):  # [D, X] -> [128, ND, X] bf16
            return np.ascontiguousarray(
                np.asarray(wmat, np.float32).astype(BF)
                .reshape(ND, 128, -1).transpose(1, 0, 2))

        w = w_all[:, b, :]  # [K, D] f32
        wT = np.ascontiguousarray(
            w.astype(BF).reshape(NT, 128, ND, 128).transpose(3, 0, 2, 1))
        rT = np.ascontiguousarray(
            r_full[:, Fs].astype(BF).T.reshape(NFC, 128, K)
            .transpose(1, 0, 2))

        m = {
            "wT": wT,
            "wres": np.ascontiguousarray(w.astype(BF)),
            "rT": rT,
            "qw": wchunk(qkv[:, 0 * H * DH:1 * H * DH][:, Fs]),
            "kw": wchunk(qkv[:, 1 * H * DH:2 * H * DH][:, Fs]),
            "vw": wchunk(qkv[:, 2 * H * DH:3 * H * DH][:, Fs]),
            "ow": ow_full,
            "ff1w": wchunk(inputs["ff1_w"][:, FFs]),
            "ff2w": np.ascontiguousarray(
                np.asarray(inputs["ff2_w"], np.float32)[FFs, :].astype(BF)
                .reshape(NFF, 128, D).transpose(1, 0, 2)),
            "rwb": np.ascontiguousarray(
                np.asarray(inputs["r_w_bias"], np.float32)[hh * HC:(hh + 1) * HC]
                .reshape(-1).reshape(NFC, 128).T),
            "rrb": np.ascontiguousarray(
                np.asarray(inputs["r_r_bias"], np.float32)[hh * HC:(hh + 1) * HC]
                .reshape(-1).reshape(NFC, 128).T),
            "ff1b": np.ascontiguousarray(
                np.asarray(inputs["ff1_b"], np.float32)[FFs]
                .reshape(NFF, 128).T),
        }
        if not _ln2_trivial(inputs):
            m["ln2w"] = np.asarray(inputs["ln2_w"], np.float32)
            m["ln2b"] = np.asarray(inputs["ln2_b"], np.float32)
        maps.append(m)
    return maps


def _ln2_trivial(inputs):
    return (np.allclose(np.asarray(inputs["ln2_w"]), 1.0)
            and np.allclose(np.asarray(inputs["ln2_b"]), 0.0))


class PjrtRunner:
    """Persistent jitted SPMD executor for a prebuilt Bass module."""

    def __init__(self, nc, n_cores=8):
        import jax
        from jax.sharding import Mesh, PartitionSpec
        from jax.experimental.shard_map import shard_map
        from concourse import mybir as _mybir
        from concourse.bass2jax import (_bass_exec_p, install_neuronx_cc_hook,
                                        partition_id_tensor)
        install_neuronx_cc_hook()
        self.jax = jax
        self.n_cores = n_cores
        in_names, out_names, out_avals = [], [], []
        partition_name = (nc.partition_id_tensor.name
                          if nc.partition_id_tensor else None)
        for alloc in nc.m.functions[0].allocations:
            if not isinstance(alloc, _mybir.MemoryLocationSet):
                continue
            name = alloc.memorylocations[0].name
            if alloc.kind == "ExternalInput":
                if name != partition_name:
                    in_names.append(name)
            elif alloc.kind == "ExternalOutput":
                out_names.append(name)
                out_avals.append(jax.core.ShapedArray(
                    tuple(alloc.tensor_shape), _mybir.dt.np(alloc.dtype)))
        self.in_names, self.out_names, self.out_avals = \
            in_names, out_names, out_avals

        def _body(*args):
            operands = list(args)
            if partition_name is not None:
                operands.append(partition_id_tensor())
            all_in = in_names + out_names
            if partition_name is not None:
                all_in = all_in + [partition_name]
            return tuple(_bass_exec_p.bind(
                *operands,
                out_avals=tuple(out_avals),
                in_names=tuple(all_in),
                out_names=tuple(out_names),
                lowering_input_output_aliases=(),
                sim_require_finite=True,
                sim_require_nnan=True,
                nc=nc,
            ))

        devices = jax.devices()[:n_cores]
        self.mesh = Mesh(np.asarray(devices), ("core",))
        nin = len(in_names) + len(out_names)
        self.fn = jax.jit(shard_map(
            _body, mesh=self.mesh,
            in_specs=(PartitionSpec("core"),) * nin,
            out_specs=(PartitionSpec("core"),) * len(out_names),
            check_rep=False))

    def pack(self, maps):
        arrs = [self.jax.device_put(
                    np.concatenate([np.asarray(maps[c][n])
                                    for c in range(self.n_cores)], axis=0))
                for n in self.in_names]
        arrs += [self.jax.device_put(
                    np.zeros((self.n_cores * a.shape[0], *a.shape[1:]),
                             a.dtype))
                 for a in self.out_avals]
        return arrs

    def __call__(self, packed):
        return self.fn(*packed)

    def unpack(self, outs):
        res = []
        for c in range(self.n_cores):
            res.append({
                n: np.asarray(outs[i]).reshape(
                    self.n_cores, *self.out_avals[i].shape)[c]
                for i, n in enumerate(self.out_names)})
        return res


def get_runner(ln2_trivial=True, debug=False):
    key = (bool(ln2_trivial),)
    if key not in _CACHED:
        nc = build(ln2_trivial=ln2_trivial)
        _CACHED[key] = PjrtRunner(nc, 8)
    return _CACHED[key]


def _assemble(inputs, results):
    ff2b = np.asarray(inputs["ff2_b"], np.float32)
    out = np.zeros((K, B, D), np.float32)
    for b in range(B):
        out[:, b, :] = (np.asarray(results[2 * b]["out"], np.float32)
                        + np.asarray(results[2 * b + 1]["out"], np.float32)
                        + ff2b[None, :])
    return out


def run(inputs, trace=False, debug=False):
    runner = get_runner(ln2_trivial=_ln2_trivial(inputs))
    maps = prep_inputs(inputs)
    packed = runner.pack(maps)
    outs = runner(packed)
    results = runner.unpack(outs)

    class R:
        pass
    res = R()
    res.results = results
    res.exec_time_ns = None
    return _assemble(inputs, results), res


def kernel(**inputs):
    inputs = {k: np.asarray(v) for k, v in inputs.items()}
    out, _ = run(inputs, trace=False, debug=False)
    return out


# revision 23
# speedup vs baseline: 1.0325x; 1.0325x over previous
"""Transformer-XL CompressiveLayer on 8 TRN2 NeuronCores.

Sharding: core c = (batch b = c//2) x (head-half hh = c%2).
Each core handles one batch's full 896 tokens with 8 of 16 heads and
2048 of 4096 FFN inner channels.

vs the earlier AllReduce design: the cores in a pair exchange the much
smaller attn_vec halves (bf16, 0.92MB) via four per-feature-chunk
AllGathers that overlap the per-head attention loop; each core then
computes the full o_proj + LN2 locally.  LN1 and the (batch-invariant)
r-projection are computed on the host, which removes the serial LN1 /
transpose prologue from the device entirely.  The BD+AC add runs on the
PE via an identity-matmul accumulate instead of a DVE tensor_add, and
the rel-shift pad write carries its zero column inside the tile (one
contiguous DMA, no strided zero fill).  FFN partial outputs are summed
on the host (plus ff2 bias).

All matmuls run in bf16 with fp32 PSUM accumulation; softmax and LN2
run in fp32.  rel_shift is realized exactly via a padded DRAM buffer
per head: BD tiles (with zero col 0) are written at flat offset
897*(128*it) and the shifted matrix is read back at flat offset
896 + 896*(128*it).
"""

import numpy as np
import ml_dtypes
from contextlib import ExitStack

import concourse.bass as bass
import concourse.tile as tile
from concourse import mybir, bacc
from concourse.masks import make_identity

F32 = mybir.dt.float32
BF16 = mybir.dt.bfloat16
BF = ml_dtypes.bfloat16

D, H, DH, FF = 1024, 16, 64, 4096
S, B, M, CM = 512, 4, 256, 128
K = S + M + CM          # 896 tokens
NT = K // 128           # 7 token tiles
ND = D // 128           # 8 D chunks
HC = 8                  # heads per core
FC = HC * DH            # 512 features per core
NFC = FC // 128         # 4 feature chunks per core
NFG = 2 * NFC           # 8 global feature chunks
FFC = FF // 2           # 2048 ffn channels per core
NFF = FFC // 128        # 16 ffn chunks per core
SCALE = 1.0 / np.sqrt(DH)
EPS = 1e-5

_CACHED = {}


def _ln_tile(nc, x_t, eps_sb, stat_pool):
    """In: x_t [128,1024] f32. Returns (mean, rstd) [128,1] tiles."""
    stats = stat_pool.tile([128, 2, 6], F32, tag="stats")
    for g in range(2):
        nc.vector.bn_stats(out=stats[:, g, :], in_=x_t[:, g * 512:(g + 1) * 512])
    mv = stat_pool.tile([128, 2], F32, tag="mv")
    nc.vector.bn_aggr(out=mv, in_=stats)
    rstd = stat_pool.tile([128, 1], F32, tag="rstd")
    nc.scalar.activation(out=rstd, in_=mv[:, 1:2],
                         func=mybir.ActivationFunctionType.Sqrt,
                         bias=eps_sb, scale=1.0)
    nc.vector.reciprocal(out=rstd, in_=rstd)
    return mv[:, 0:1], rstd


def build(ln2_trivial=True, debug=False, sim_mode=False):
    nc = bacc.Bacc(None)

    wT_p = nc.declare_dram_parameter("wT", [128, NT, ND, 128], BF16,
                                     isOutput=False)
    wres_p = nc.declare_dram_parameter("wres", [K, D], BF16, isOutput=False)
    rT_p = nc.declare_dram_parameter("rT", [128, NFC, K], BF16, isOutput=False)
    qw = nc.declare_dram_parameter("qw", [128, ND, FC], BF16, isOutput=False)
    kw = nc.declare_dram_parameter("kw", [128, ND, FC], BF16, isOutput=False)
    vw = nc.declare_dram_parameter("vw", [128, ND, FC], BF16, isOutput=False)
    ow = nc.declare_dram_parameter("ow", [128, NFG, D], BF16, isOutput=False)
    ff1w = nc.declare_dram_parameter("ff1w", [128, ND, FFC], BF16,
                                     isOutput=False)
    ff2w = nc.declare_dram_parameter("ff2w", [128, NFF, D], BF16,
                                     isOutput=False)
    rwb = nc.declare_dram_parameter("rwb", [128, NFC], F32, isOutput=False)
    rrb = nc.declare_dram_parameter("rrb", [128, NFC], F32, isOutput=False)
    ff1b = nc.declare_dram_parameter("ff1b", [128, NFF], F32, isOutput=False)
    if not ln2_trivial:
        ln2w = nc.declare_dram_parameter("ln2w", [D], F32, isOutput=False)
        ln2b = nc.declare_dram_parameter("ln2b", [D], F32, isOutput=False)

    out = nc.declare_dram_parameter("out", [K, D], F32, isOutput=True)

    pads = [nc.dram_tensor(f"pad{i}", [K * (K + 1)], BF16) for i in range(HC)]
    ag_in = nc.dram_tensor("ag_in", [NFC * 128 * K], BF16)
    ag_out = nc.dram_tensor("ag_out", [2 * NFC * 128 * K], BF16)

    with tile.TileContext(nc) as tc, ExitStack() as ctx:
        consts = ctx.enter_context(tc.tile_pool(name="consts", bufs=1))
        psS = ctx.enter_context(tc.tile_pool(name="psS", bufs=2, space="PSUM"))
        stat_pool = ctx.enter_context(tc.tile_pool(name="stats", bufs=8))

        eps_sb = consts.tile([128, 1], F32)
        nc.vector.memset(eps_sb, EPS)
        ident = consts.tile([128, 128], BF16, tag="ident")
        make_identity(nc, ident)
        rwb_sb = consts.tile([128, NFC], F32, tag="rwb")
        nc.sync.dma_start(out=rwb_sb, in_=rwb[:])
        rrb_sb = consts.tile([128, NFC], F32, tag="rrb")
        nc.sync.dma_start(out=rrb_sb, in_=rrb[:])
        ff1b_sb = consts.tile([128, NFF], F32, tag="ff1b")
        nc.sync.dma_start(out=ff1b_sb, in_=ff1b[:])
        if not ln2_trivial:
            def bcast(name, param):
                t = consts.tile([128, D], F32, tag=name)
                nc.sync.dma_start(out=t, in_=bass.AP(tensor=param, offset=0,
                                                     ap=[[0, 128], [1, D]]))
                return t
            ln2w_bc = bcast("ln2w_bc", ln2w)
            ln2b_bc = bcast("ln2b_bc", ln2b)

        # late weights: pW1 lives from the start; pW2 is created only after
        # the attention pools free their SBUF
        pW1 = ctx.enter_context(tc.tile_pool(name="pW1", bufs=1))
        ow_sb = pW1.tile([128, NFG, D], BF16, tag="ow")
        wres_sb = pW1.tile([128, NT, D], BF16, tag="wres")
        ff1w_sb = pW1.tile([128, ND, FFC], BF16, tag="ff1w")

        avp = ctx.enter_context(tc.tile_pool(name="avp", bufs=1))
        avA = [avp.tile([128, K], BF16, tag=f"av{f}", name=f"av{f}")
               for f in range(NFG)]

        # persistent activations for attention (closed before pW2 is created)
        atn = ExitStack()
        atnp = atn.enter_context(tc.tile_pool(name="atnp", bufs=1))
        q1T = atnp.tile([128, NFC, K], BF16, tag="q1T")
        q2T = atnp.tile([128, NFC, K], BF16, tag="q2T")
        kT = atnp.tile([128, NFC, K], BF16, tag="kT")
        rT_sb = atnp.tile([128, NFC, K], BF16, tag="rT")
        v1 = atnp.tile([128, NT, HC, DH], BF16, tag="v1")

        # ---------------- P0: load weights, projections ----------------
        with tc.tile_pool(name="p0w", bufs=1) as p0w:
            wT = p0w.tile([128, NT, ND, 128], BF16, tag="wT")
            nc.sync.dma_start(out=wT, in_=wT_p[:])
            qw_sb = p0w.tile([128, ND, FC], BF16, tag="qw")
            nc.scalar.dma_start(out=qw_sb, in_=qw[:])
            kw_sb = p0w.tile([128, ND, FC], BF16, tag="kw")
            nc.scalar.dma_start(out=kw_sb, in_=kw[:])
            vw_sb = p0w.tile([128, ND, FC], BF16, tag="vw")
            nc.scalar.dma_start(out=vw_sb, in_=vw[:])
            nc.scalar.dma_start(out=rT_sb, in_=rT_p[:])
            # o_w / residual loads, split to avoid long DMA-engine holds
            for fcg in range(NFG):
                nc.scalar.dma_start(out=ow_sb[:, fcg, :], in_=ow[:, fcg, :])
            for t in range(NT):
                nc.scalar.dma_start(
                    out=wres_sb[:, t, :],
                    in_=wres_p[t * 128:(t + 1) * 128, :])

            def projT(w_sb, writeback):
                for fc in range(NFC):
                    ps = psS.tile([128, 1024], F32, tag="s", name=f"psp{fc}")
                    for dc in range(ND):
                        st = dict(start=(dc == 0), stop=(dc == ND - 1))
                        lh = w_sb[:, dc, fc * 128:(fc + 1) * 128]
                        nc.tensor.matmul(ps[:, 0:512], lh, wT[:, 0:4, dc, :],
                                         **st)
                        nc.tensor.matmul(ps[:, 512:896], lh, wT[:, 4:7, dc, :],
                                         **st)
                    writeback(fc, ps)

            def q_wb(fc, ps):
                nc.vector.tensor_scalar_add(
                    out=q1T[:, fc, :], in0=ps[:, 0:K],
                    scalar1=rwb_sb[:, fc:fc + 1])
                nc.vector.tensor_scalar_add(
                    out=q2T[:, fc, :], in0=ps[:, 0:K],
                    scalar1=rrb_sb[:, fc:fc + 1])

            def k_wb(fc, ps):
                eng = nc.scalar.copy if fc % 2 else nc.vector.tensor_copy
                eng(out=kT[:, fc, :], in_=ps[:, 0:K])

            projT(qw_sb, q_wb)
            projT(kw_sb, k_wb)

            for t in range(NT):
                pv = psS.tile([128, 1024], F32, tag="s", name="psv")
                for dc in range(ND):
                    nc.tensor.matmul(pv[:, 0:512], wT[:, t, dc, :],
                                     vw_sb[:, dc, :],
                                     start=(dc == 0), stop=(dc == ND - 1))
                eng = nc.scalar.copy if t % 2 else nc.vector.tensor_copy
                eng(out=v1[:, t, :, :],
                    in_=pv[:, 0:512].rearrange("p (h d) -> p h d", h=HC))

        # ---------------- P1: per-head attention + avT exchange ----------
        # Software pipeline: PV of head h-1 is emitted between the BD and
        # AC/softmax phases of head h so its matmuls fill the pad-DMA and
        # softmax latency; the pair AllGather and its imports ride the
        # otherwise idle gpsimd queue so no compute queue waits on the
        # collective.
        with tc.tile_pool(name="pBD", bufs=3) as pBD, \
             tc.tile_pool(name="pSBD", bufs=9) as pSBD, \
             tc.tile_pool(name="pPX", bufs=3) as pPX, \
             tc.tile_pool(name="ptwp", bufs=2) as ptwp, \
             tc.tile_pool(name="psPV", bufs=1, space="PSUM") as psPV:
            ptws = {}

            def bd_step(h, it):
                hp, fc = (h % 2) * DH, h // 2
                ps = psS.tile([128, 1024], F32, tag="s", name="psbd")
                nc.tensor.matmul(
                    ps[:, 0:512],
                    q2T[hp:hp + DH, fc, it * 128:(it + 1) * 128],
                    rT_sb[hp:hp + DH, fc, 0:512], start=True, stop=True)
                nc.tensor.matmul(
                    ps[:, 512:896],
                    q2T[hp:hp + DH, fc, it * 128:(it + 1) * 128],
                    rT_sb[hp:hp + DH, fc, 512:896], start=True, stop=True)
                bdt = pBD.tile([128, K + 1], BF16, tag="bd", name="bd")
                nc.vector.memset(bdt[:, 0:1], 0.0)
                eng = nc.scalar.copy if it % 2 else nc.vector.tensor_copy
                eng(out=bdt[:, 1:K + 1], in_=ps[:, 0:K])
                nc.sync.dma_start(
                    out=bass.AP(tensor=pads[h],
                                offset=(K + 1) * 128 * it,
                                ap=[[K + 1, 128], [1, K + 1]]),
                    in_=bdt)

            def s_read(h, it):
                sbd = pSBD.tile([128, K], BF16, tag="sbd", name="sbd")
                nc.sync.dma_start(
                    out=sbd,
                    in_=bass.AP(tensor=pads[h], offset=K + K * 128 * it,
                                ap=[[K, 128], [1, K]]))
                return sbd

            def s_step(h, it, sbd):
                hp, fc = (h % 2) * DH, h // 2
                ps = psS.tile([128, 1024], F32, tag="s", name="psac")
                nc.tensor.matmul(
                    ps[:, 0:512],
                    q1T[hp:hp + DH, fc, it * 128:(it + 1) * 128],
                    kT[hp:hp + DH, fc, 0:512], start=True, stop=False)
                nc.tensor.matmul(
                    ps[:, 0:512], ident, sbd[:, 0:512],
                    start=False, stop=True)
                nc.tensor.matmul(
                    ps[:, 512:896],
                    q1T[hp:hp + DH, fc, it * 128:(it + 1) * 128],
                    kT[hp:hp + DH, fc, 512:896], start=True, stop=False)
                nc.tensor.matmul(
                    ps[:, 512:896], ident, sbd[:, 512:896],
                    start=False, stop=True)
                pexp = pPX.tile([128, K], BF16, tag="pexp", name="pexp")
                acc = stat_pool.tile([128, 1], F32, tag="acc", name="acc")
                nc.scalar.activation(
                    out=pexp, in_=ps[:, 0:K],
                    func=mybir.ActivationFunctionType.Exp,
                    scale=float(SCALE), accum_out=acc)
                rcp = stat_pool.tile([128, 1], F32, tag="rcp", name="rcp")
                nc.vector.reciprocal(out=rcp, in_=acc)
                nc.vector.tensor_scalar_mul(out=pexp, in0=pexp,
                                            scalar1=rcp)
                nc.sync.dma_start_transpose(ptws[h][:, it, :, :], pexp)

            pv_state = {}

            def pv_start(h):
                pv_state[h] = (ptws.pop(h),
                               psPV.tile([64, 1024], F32, tag="pv",
                                         name="pvx"))

            def pv_step(h, jt):
                ptw, pvx = pv_state[h]
                st = dict(start=(jt == 0), stop=(jt == NT - 1))
                nc.tensor.matmul(pvx[:, 0:512], v1[:, jt, h, :],
                                 ptw[:, 0:4, jt, :], **st)
                nc.tensor.matmul(pvx[:, 512:896], v1[:, jt, h, :],
                                 ptw[:, 4:7, jt, :], **st)

            def pv_finish(h):
                hp, fc = (h % 2) * DH, h // 2
                _, pvx = pv_state.pop(h)
                eng = nc.scalar.copy if h % 2 else nc.vector.tensor_copy
                eng(out=avA[fc][hp:hp + DH, 0:512], in_=pvx[:, 0:512])
                eng(out=avA[fc][hp:hp + DH, 512:K], in_=pvx[:, 512:896])

            def pv(h):
                pv_start(h)
                for jt in range(NT):
                    pv_step(h, jt)
                pv_finish(h)

            def store_chunk(f):
                # export own chunk f to the AllGather staging buffer
                nc.sync.dma_start(
                    out=bass.AP(tensor=ag_in, offset=f * 128 * K,
                                ap=[[K, 128], [1, K]]),
                    in_=avA[f])

            def exchange_all():
                # one AllGather of all four chunks with the pair partner,
                # then import rank0's block into global slots 0-3 and
                # rank1's into 4-7 (symmetric on both cores)
                if sim_mode:
                    for r in range(2):
                        nc.gpsimd.dma_start(
                            out=bass.AP(tensor=ag_out,
                                        offset=r * NFC * 128 * K,
                                        ap=[[1, NFC * 128 * K]]),
                            in_=bass.AP(tensor=ag_in, offset=0,
                                        ap=[[1, NFC * 128 * K]]))
                else:
                    nc.gpsimd.collective_compute(
                        "AllGather", mybir.AluOpType.bypass,
                        replica_groups=[[0, 1], [2, 3], [4, 5], [6, 7]],
                        ins=[bass.AP(tensor=ag_in, offset=0,
                                     ap=[[1, NFC * 128 * K]])],
                        outs=[bass.AP(tensor=ag_out, offset=0,
                                      ap=[[1, 2 * NFC * 128 * K]])])
                for g in range(NFG):
                    eng = nc.sync if g % 2 else nc.scalar
                    eng.dma_start(
                        out=avA[g],
                        in_=bass.AP(tensor=ag_out, offset=g * 128 * K,
                                    ap=[[K, 128], [1, K]]))

            for h in range(HC):
                ptws[h] = ptwp.tile([128, NT, NT, 128], BF16, tag="ptw",
                                    name="ptw")
                reads = {}
                bd_step(h, 0)
                for it in range(1, NT):
                    bd_step(h, it)
                    reads[it - 1] = s_read(h, it - 1)
                reads[NT - 1] = s_read(h, NT - 1)
                if h >= 1:
                    pv(h - 1)
                for it in range(NT):
                    s_step(h, it, reads[it])
                if h >= 2 and h % 2 == 0:
                    store_chunk(h // 2 - 1)
                if h == 6:
                    for dc in range(ND):
                        nc.scalar.dma_start(out=ff1w_sb[:, dc, :],
                                            in_=ff1w[:, dc, :])
            pv(HC - 1)
            store_chunk(NFC - 1)
            exchange_all()

        atn.close()
        # ---------------- P2: o_proj + LN2 ----------------
        pW2 = ctx.enter_context(tc.tile_pool(name="pW2", bufs=1))
        ff2w_sb = pW2.tile([128, NFF, D], BF16, tag="ff2w")
        for ffc in range(NFF):
            nc.scalar.dma_start(out=ff2w_sb[:, ffc, :], in_=ff2w[:, ffc, :])
        hT = pW2.tile([128, NFF, K], BF16, tag="hT")
        arT = pW2.tile([128, NT, ND, 128], BF16, tag="arT")

        with tc.tile_pool(name="p2", bufs=2) as p2:
            for it in range(NT):
                po = psS.tile([128, 1024], F32, tag="s", name="po")
                for fcg in range(NFG):
                    st = dict(start=(fcg == 0), stop=(fcg == NFG - 1))
                    l = avA[fcg][:, it * 128:(it + 1) * 128]
                    nc.tensor.matmul(po[:, 0:512], l, ow_sb[:, fcg, 0:512],
                                     **st)
                    nc.tensor.matmul(po[:, 512:1024], l,
                                     ow_sb[:, fcg, 512:1024], **st)
                x2 = p2.tile([128, D], F32, tag="x2")
                nc.vector.tensor_add(out=x2, in0=po, in1=wres_sb[:, it, :])
                mean, rstd = _ln_tile(nc, x2, eps_sb, stat_pool)
                ars = p2.tile([128, D], BF16, tag="ars")
                if ln2_trivial:
                    nc.vector.tensor_scalar(out=ars, in0=x2, scalar1=mean,
                                            scalar2=rstd,
                                            op0=mybir.AluOpType.subtract,
                                            op1=mybir.AluOpType.mult)
                else:
                    nc.vector.tensor_scalar(out=x2, in0=x2, scalar1=mean,
                                            scalar2=rstd,
                                            op0=mybir.AluOpType.subtract,
                                            op1=mybir.AluOpType.mult)
                    nc.gpsimd.tensor_mul(out=x2, in0=x2, in1=ln2w_bc)
                    nc.vector.tensor_add(out=ars, in0=x2, in1=ln2b_bc)
                nc.sync.dma_start_transpose(arT[:, it, :, :], ars)


        # ---------------- P3: FFN ----------------
        with tc.tile_pool(name="psB", bufs=2, space="PSUM") as psB:
            for half, (i0, n, sl) in enumerate(
                    [(0, 512, (0, 4)), (512, 384, (4, 7))]):
                for ffc in range(NFF):
                    ph = psB.tile([128, 512], F32, tag="h", name="ph")
                    for dc in range(ND):
                        st = dict(start=(dc == 0), stop=(dc == ND - 1))
                        lh = ff1w_sb[:, dc, ffc * 128:(ffc + 1) * 128]
                        nc.tensor.matmul(ph[:, 0:n], lh,
                                         arT[:, sl[0]:sl[1], dc, :], **st)
                    nc.scalar.activation(
                        out=hT[:, ffc, i0:i0 + n], in_=ph[:, 0:n],
                        func=mybir.ActivationFunctionType.Relu,
                        bias=ff1b_sb[:, ffc:ffc + 1], scale=1.0)
            with tc.tile_pool(name="p3", bufs=3) as p3:
                for it in range(NT):
                    po = psS.tile([128, 1024], F32, tag="s", name="pf")
                    for ffc in range(NFF):
                        st = dict(start=(ffc == 0), stop=(ffc == NFF - 1))
                        l = hT[:, ffc, it * 128:(it + 1) * 128]
                        nc.tensor.matmul(po[:, 0:512], l,
                                         ff2w_sb[:, ffc, 0:512], **st)
                        nc.tensor.matmul(po[:, 512:1024], l,
                                         ff2w_sb[:, ffc, 512:1024], **st)
                    ot = p3.tile([128, D], F32, tag="ot")
                    eng = nc.scalar.copy if it % 2 else nc.vector.tensor_copy
                    eng(out=ot, in_=po)
                    nc.sync.dma_start(out=out[it * 128:(it + 1) * 128, :],
                                      in_=ot)

    nc.finalize()
    return nc


def prep_inputs(inputs):
    """Full inputs -> list of 8 per-core input maps (host does LN1 + r-proj
    + all layout transposes)."""
    x_nat = np.concatenate([inputs["input_ids"], inputs["mem"],
                            inputs["c_mem"]], axis=0)  # [K,B,D] f32
    x_nat = np.asarray(x_nat, np.float32)
    mu = x_nat.mean(-1, keepdims=True)
    var = x_nat.var(-1, keepdims=True)
    w_all = ((x_nat - mu) / np.sqrt(var + EPS)
             * np.asarray(inputs["ln1_w"], np.float32)
             + np.asarray(inputs["ln1_b"], np.float32))  # [K,B,D] f32

    r_full = (np.asarray(inputs["positional_embedding"], np.float32)
              @ np.asarray(inputs["r_w"], np.float32))  # [K, H*DH] f32

    qkv = inputs["qkv_w"]
    ow_full = np.ascontiguousarray(
        np.asarray(inputs["o_w"], np.float32).astype(BF)
        .reshape(NFG, 128, D).transpose(1, 0, 2))

    maps = []
    for c in range(8):
        b, hh = c // 2, c % 2
        Fs = slice(hh * FC, (hh + 1) * FC)
        FFs = slice(hh * FFC, (hh + 1) * FFC)

        def wchunk(wmat):  # [D, X] -> [128, ND, X] bf16
            return np.ascontiguousarray(
                np.asarray(wmat, np.float32).astype(BF)
                .reshape(ND, 128, -1).transpose(1, 0, 2))

        w = w_all[:, b, :]  # [K, D] f32
        wT = np.ascontiguousarray(
            w.astype(BF).reshape(NT, 128, ND, 128).transpose(3, 0, 2, 1))
        rT = np.ascontiguousarray(
            r_full[:, Fs].astype(BF).T.reshape(NFC, 128, K)
            .transpose(1, 0, 2))

        m = {
            "wT": wT,
            "wres": np.ascontiguousarray(w.astype(BF)),
            "rT": rT,
            "qw": wchunk(qkv[:, 0 * H * DH:1 * H * DH][:, Fs]),
            "kw": wchunk(qkv[:, 1 * H * DH:2 * H * DH][:, Fs]),
            "vw": wchunk(qkv[:, 2 * H * DH:3 * H * DH][:, Fs]),
            "ow": ow_full,
            "ff1w": wchunk(inputs["ff1_w"][:, FFs]),
            "ff2w": np.ascontiguousarray(
                np.asarray(inputs["ff2_w"], np.float32)[FFs, :].astype(BF)
                .reshape(NFF, 128, D).transpose(1, 0, 2)),
            "rwb": np.ascontiguousarray(
                np.asarray(inputs["r_w_bias"], np.float32)[hh * HC:(hh + 1) * HC]
                .reshape(-1).reshape(NFC, 128).T),
            "rrb": np.ascontiguousarray(
                np.asarray(inputs["r_r_bias"], np.float32)[hh * HC:(hh + 1) * HC]
                .reshape(-1).reshape(NFC, 128).T),
            "ff1b": np.ascontiguousarray(
                np.asarray(inputs["ff1_b"], np.float32)[FFs]
                .reshape(NFF, 128).T),
        }
        if not _ln2_trivial(inputs):
            m["ln2w"] = np.asarray(inputs["ln2_w"], np.float32)
            m["ln2b"] = np.asarray(inputs["ln2_b"], np.float32)
        maps.append(m)
    return maps


def _ln2_trivial(inputs):
    return (np.allclose(np.asarray(inputs["ln2_w"]), 1.0)
            and np.allclose(np.asarray(inputs["ln2_b"]), 0.0))


class PjrtRunner:
    """Persistent jitted SPMD executor for a prebuilt Bass module."""

    def __init__(self, nc, n_cores=8):
        import jax
        from jax.sharding import Mesh, PartitionSpec
        from jax.experimental.shard_map import shard_map
        from concourse import mybir as _mybir
        from concourse.bass2jax import (_bass_exec_p, install_neuronx_cc_hook,
                                        partition_id_tensor)
        install_neuronx_cc_hook()
        self.jax = jax
        self.n_cores = n_cores
        in_names, out_names, out_avals = [], [], []
        partition_name = (nc.partition_id_tensor.name
                          if nc.partition_id_tensor else None)
        for alloc in nc.m.functions[0].allocations:
            if not isinstance(alloc, _mybir.MemoryLocationSet):
                continue
            name = alloc.memorylocations[0].name
            if alloc.kind == "ExternalInput":
                if name != partition_name:
                    in_names.append(name)
            elif alloc.kind == "ExternalOutput":
                out_names.append(name)
                out_avals.append(jax.core.ShapedArray(
                    tuple(alloc.tensor_shape), _mybir.dt.np(alloc.dtype)))
        self.in_names, self.out_names, self.out_avals = \
            in_names, out_names, out_avals

        def _body(*args):
            operands = list(args)
            if partition_name is not None:
                operands.append(partition_id_tensor())
            all_in = in_names + out_names
            if partition_name is not None:
                all_in = all_in + [partition_name]
            return tuple(_bass_exec_p.bind(
                *operands,
                out_avals=tuple(out_avals),
                in_names=tuple(all_in),
                out_names=tuple(out_names),
                lowering_input_output_aliases=(),
                sim_require_finite=True,
                sim_require_nnan=True,
                nc=nc,
            ))

        devices = jax.devices()[:n_cores]
        self.mesh = Mesh(np.asarray(devices), ("core",))
        nin = len(in_names) + len(out_names)
        self.fn = jax.jit(shard_map(
            _body, mesh=self.mesh,
            in_specs=(PartitionSpec("core"),) * nin,
            out_specs=(PartitionSpec("core"),) * len(out_names),
            check_rep=False))

    def pack(self, maps):
        arrs = [self.jax.device_put(
                    np.concatenate([np.asarray(maps[c][n])
                                    for c in range(self.n_cores)], axis=0))
                for n in self.in_names]
        arrs += [self.jax.device_put(
                    np.zeros((self.n_cores * a.shape[0], *a.shape[1:]),
                             a.dtype))
                 for a in self.out_avals]
        return arrs

    def __call__(self, packed):
        return self.fn(*packed)

    def unpack(self, outs):
        res = []
        for c in range(self.n_cores):
            res.append({
                n: np.asarray(outs[i]).reshape(
                    self.n_cores, *self.out_avals[i].shape)[c]
                for i, n in enumerate(self.out_names)})
        return res


def get_runner(ln2_trivial=True, debug=False):
    key = (bool(ln2_trivial),)
    if key not in _CACHED:
        nc = build(ln2_trivial=ln2_trivial)
        _CACHED[key] = PjrtRunner(nc, 8)
    return _CACHED[key]


def _assemble(inputs, results):
    ff2b = np.asarray(inputs["ff2_b"], np.float32)
    out = np.zeros((K, B, D), np.float32)
    for b in range(B):
        out[:, b, :] = (np.asarray(results[2 * b]["out"], np.float32)
                        + np.asarray(results[2 * b + 1]["out"], np.float32)
                        + ff2b[None, :])
    return out


def run(inputs, trace=False, debug=False):
    runner = get_runner(ln2_trivial=_ln2_trivial(inputs))
    maps = prep_inputs(inputs)
    packed = runner.pack(maps)
    outs = runner(packed)
    results = runner.unpack(outs)

    class R:
        pass
    res = R()
    res.results = results
    res.exec_time_ns = None
    return _assemble(inputs, results), res


def kernel(**inputs):
    inputs = {k: np.asarray(v) for k, v in inputs.items()}
    out, _ = run(inputs, trace=False, debug=False)
    return out
